# revision 3
# baseline (speedup 1.0000x reference)
"""nms_detection (HardDetectionModule) Trainium2 Bass kernel.

Reference semantics (per batch sample):
    M    = max over channels of x                      # [H, W]
    L_c  = 9x9 sliding max (stride 1, -inf pad) of x_c # [C, H, W]
    prob = sum_c where(x_c == M and x_c == L_c, x_c, 0)

Since every passing channel satisfies x_c == M, prob == M * count with
count = #{c : x_c >= max(L_c, M)} (exact, including ties).

Sharding: pure data parallel, B(4) x H-halves(2) -> 8 cores. Host pads each
shard with -inf to [256, 108, 328] so one SPMD program covers all cores and
no in-kernel boundary handling is needed.

Per-core program (channels on partitions, 2 groups of 128):
  - 10 W-strips of 32 output cols (40 input cols with halo)
  - sliding 9-max via log-tree: 4 tensor_tensor(max) ops per direction
  - M via gpsimd.partition_all_reduce(max) (result broadcast to all parts)
  - cmp = is_ge(x, max(h9, Mb)) -> bf16 0/1
  - count = ones[128,1]^T @ cmp on TensorE (PSUM accumulate over both groups)
  - prob = count * M, DMA out
"""

import os
import sys
import numpy as np
from contextlib import ExitStack

sys.path.insert(0, "/opt/trn_rl_repo")

import concourse.bacc as bacc
import concourse.tile as tile
from concourse import mybir, bass_isa
from concourse.bass_utils import run_bass_kernel_spmd

F32 = mybir.dt.float32
BF16 = mybir.dt.bfloat16

B, C, H, W = 4, 256, 200, 320
HS, WS = 108, 328          # padded slab dims per core
OH, OW = 100, 320          # per-core output
NSTRIP, SW, SWIN = 10, 32, 40  # strips of 32 out cols, 40 in cols

_PROGRAM = None


def build_program():
    nc = bacc.Bacc("TRN2", target_bir_lowering=False, debug=False, num_devices=8)
    x_ap = nc.dram_tensor("x", [C, HS, WS], F32, kind="ExternalInput").ap()
    o_ap = nc.dram_tensor("out", [OH, OW], F32, kind="ExternalOutput").ap()

    mx = mybir.AluOpType.max
    ge = mybir.AluOpType.is_ge
    mul = mybir.AluOpType.mult

    with tile.TileContext(nc) as tc, ExitStack() as ctx:
        xp = ctx.enter_context(tc.tile_pool(name="xg", bufs=3))
        tmp = ctx.enter_context(tc.tile_pool(name="tmp", bufs=2))
        w9p = ctx.enter_context(tc.tile_pool(name="w9", bufs=2))
        h9p = ctx.enter_context(tc.tile_pool(name="h9", bufs=3))
        mp = ctx.enter_context(tc.tile_pool(name="m", bufs=1))
        mbp = ctx.enter_context(tc.tile_pool(name="mb", bufs=1))
        cp = ctx.enter_context(tc.tile_pool(name="cmp", bufs=2))
        pp = ctx.enter_context(tc.tile_pool(name="prob", bufs=1))
        onep = ctx.enter_context(tc.tile_pool(name="ones", bufs=1))
        psp = ctx.enter_context(tc.tile_pool(name="psum", bufs=2, space="PSUM"))

        ones = onep.tile([128, 1], BF16, name="ones")
        nc.gpsimd.memset(ones[:], 1.0)

        for s in range(NSTRIP):
            c0 = s * SW
            xg = []
            for g in range(2):
                xt = xp.tile([128, HS, SWIN], F32, tag="xg", name=f"x_{s}_{g}")
                nc.sync.dma_start(xt[:], x_ap[g * 128:(g + 1) * 128, :, c0:c0 + SWIN])
                xg.append(xt)

            h9s = []
            for g in range(2):
                x_ = xg[g]
                w2 = tmp.tile([128, HS, 39], F32, tag="tmp", name=f"w2_{s}_{g}")
                nc.vector.tensor_tensor(w2[:], x_[:, :, 0:39], x_[:, :, 1:40], mx)
                w4 = tmp.tile([128, HS, 37], F32, tag="tmp", name=f"w4_{s}_{g}")
                nc.vector.tensor_tensor(w4[:], w2[:, :, 0:37], w2[:, :, 2:39], mx)
                w8 = tmp.tile([128, HS, 33], F32, tag="tmp", name=f"w8_{s}_{g}")
                nc.vector.tensor_tensor(w8[:], w4[:, :, 0:33], w4[:, :, 4:37], mx)
                w9 = w9p.tile([128, HS, SW], F32, tag="w9", name=f"w9_{s}_{g}")
                nc.vector.tensor_tensor(w9[:], w8[:, :, 0:32], x_[:, :, 8:40], mx)
                h2 = tmp.tile([128, 107, SW], F32, tag="tmp", name=f"h2_{s}_{g}")
                nc.vector.tensor_tensor(h2[:], w9[:, 0:107, :], w9[:, 1:108, :], mx)
                h4 = tmp.tile([128, 105, SW], F32, tag="tmp", name=f"h4_{s}_{g}")
                nc.vector.tensor_tensor(h4[:], h2[:, 0:105, :], h2[:, 2:107, :], mx)
                h8 = tmp.tile([128, 101, SW], F32, tag="tmp", name=f"h8_{s}_{g}")
                nc.vector.tensor_tensor(h8[:], h4[:, 0:101, :], h4[:, 4:105, :], mx)
                h9 = h9p.tile([128, OH, SW], F32, tag="h9", name=f"h9_{s}_{g}")
                nc.vector.tensor_tensor(h9[:], h8[:, 0:100, :], w9[:, 8:108, :], mx)
                h9s.append(h9)

            xc = [xg[g][:, 4:104, 4:36] for g in range(2)]
            m = mp.tile([128, OH, SW], F32, tag="m", name=f"m_{s}")
            meng = nc.gpsimd if OFFLOAD_M else nc.vector
            meng.tensor_tensor(m[:], xc[0], xc[1], mx)
            mb = mbp.tile([128, OH, SW], F32, tag="mb", name=f"mb_{s}")
            nc.gpsimd.partition_all_reduce(mb[:], m[:], 128, bass_isa.ReduceOp.max)

            ceng = nc.gpsimd if OFFLOAD_CMP else nc.vector
            cmps = []
            for g in range(2):
                gt = h9p.tile([128, OH, SW], F32, tag="h9", name=f"g_{s}_{g}")
                ceng.tensor_tensor(gt[:], h9s[g][:], mb[:], mx)
                cb = cp.tile([128, OH, SW], BF16, tag="cmp", name=f"cb_{s}_{g}")
                ceng.tensor_tensor(cb[:], xc[g], gt[:], ge)
                cmps.append(cb)

            prob = pp.tile([1, OH, SW], F32, tag="prob", name=f"prob_{s}")
            cnt = psp.tile([1, OH, SW], F32, tag="cnt", name=f"cnt_{s}")
            for r0 in range(0, OH, 16):
                r1 = min(r0 + 16, OH)
                nc.tensor.matmul(cnt[:, r0:r1, :], ones[:], cmps[0][:, r0:r1, :],
                                 start=True, stop=False)
                nc.tensor.matmul(cnt[:, r0:r1, :], ones[:], cmps[1][:, r0:r1, :],
                                 start=False, stop=True)
            nc.vector.tensor_tensor(prob[:], cnt[:], mb[0:1, :, :], mul)
            nc.sync.dma_start(o_ap[:, c0:c0 + SW], prob[:])

    nc.compile()
    return nc


def get_program():
    global _PROGRAM
    if _PROGRAM is None:
        _PROGRAM = build_program()
    return _PROGRAM


def shard_input(x):
    """x: [4, 256, 200, 320] -> 8 slabs [256, 108, 328], -inf padded."""
    slabs = []
    for core in range(8):
        b, half = core // 2, core % 2
        slab = np.full((C, HS, WS), -np.inf, dtype=np.float32)
        r0 = half * 100 - 4
        lo, hi = max(r0, 0), min(r0 + HS, H)
        slab[:, lo - r0:hi - r0, 4:4 + W] = x[b, :, lo:hi, :]
        slabs.append(slab)
    return slabs


def kernel(x):
    x = np.ascontiguousarray(x, dtype=np.float32)
    assert x.shape == (B, C, H, W), x.shape
    nc = get_program()
    in_maps = [{"x": slab} for slab in shard_input(x)]
    res = run_bass_kernel_spmd(nc, in_maps, list(range(8))).results
    out = np.empty((B, H, W), np.float32)
    for core in range(8):
        b, half = core // 2, core % 2
        out[b, half * 100:(half + 1) * 100, :] = res[core]["out"].reshape(OH, OW)
    return out
